# revision 28
# baseline (speedup 1.0000x reference)
"""Attention graph convolution (GAT layer) on 8 TRN2 NeuronCores.

Reference computation (all fp32):
    h   = input @ W                      # (N, 64)
    e   = leakyrelu(h@a1 + (h@a2).T)     # (N, N)
    att = softmax(where(adj>0, e, -inf)) # row softmax
    out = elu(att @ h)                   # (N, 64)

Sharding: rows of e/att (= output rows) are split across 8 cores,
1536 rows each.  h (N x 64) is computed on every core (tiny).

Host-side prep: input transposed + cast bf16; adjacency row-block
transposed and expanded to an int16 AND-mask (adj=1 -> 0xFFFF,
adj=0 -> 0x0000).  2-byte masks keep every DVE op in its fast mode
(1-byte operands drop DVE to 1x) and plain (non-cast) DMAs run ~2x
faster per byte than the cast DMAs the int32 layout needed.

Per-core algorithm (core owns rows I, |I| = 1536; layout [j part, i free]):
  - m = S*leakyrelu(Wh1[i] + Wh2[j]) via one ACT Prelu per j-chunk
    (scale=S, per-partition bias = S*Wh2), S = (2^7/ln2)/127.
  - exp via the Schraudolph bit trick: one 4x DVE tensor_scalar computes
    bits = round((m + 127.953)*127) into int16; those bits reinterpreted
    as bf16 ARE exp(leakyrelu(z)) up to a +-4% sawtooth that cancels in
    the row softmax (normed output err ~5e-3, sim+HW validated).  No
    exact exp in the main loop.
  - mask applied in the bits domain: bitwise AND with the int16 mask
    (2x tensor_tensor on DVE, a slice of chunks on the otherwise-idle
    gpsimd).  Masked bits collapse to < 2^-120 in bf16 -- exact enough.
  - all elementwise work happens in the window prep stage; the consume
    stage is a pure PE burst (24 bf16 matmuls accumulating [h|1].T @ U.T
    into f32 psum), which keeps the PE HAM clock-gate warm.
  - denominator via ones-column; out = elu(P[:, :64] / P[:, 64]).
"""

import numpy as np

N_TOTAL = 12288
K_IN = 128
F_OUT = 64
N_CORES = 8
ALPHA = 0.2
KS = 184.662545               # 2^7/ln2: bits-scale of lrelu(z)
BADD = 16248.0                # fp16-exact bits offset (127*2^7 - corr)
BOFF = 1100.0                 # masked-entry offset -> u < 2^-90
JW = 1024


def build_program(
    nt: int,          # total nodes
    no: int,          # nodes owned by this core
    jw: int,          # j window size
    act_frac: float = 0.98,   # fraction of j-chunk PAIRS with leakyrelu on ACT
    gps_frac: float = 0.20,   # fraction of pairs whose exp+mask runs on gpsimd
    ragged: bool = True,
):
    from contextlib import ExitStack

    import concourse.bass as bass
    import concourse.mybir as mybir
    import concourse.tile as tile
    from concourse import bacc
    from concourse.alu_op_type import AluOpType

    f32 = mybir.dt.float32
    i16 = mybir.dt.int16
    bf16 = mybir.dt.bfloat16
    fp16 = mybir.dt.float16
    AF = mybir.ActivationFunctionType

    P = 128
    F = F_OUT
    FE = F + 1                    # [wh2 | h] phase-1b columns
    FE2 = FE + 1                  # h_ext columns: [wh2 | h | ones]
    K = K_IN
    assert nt % P == 0 and no % P == 0 and jw % P == 0 and nt % jw == 0
    ncj = nt // P                 # j chunks
    cpw = jw // P                 # j chunks per full window
    nic = no // P                 # i chunks (own rows)
    S = 512                       # psum bank split of the i dim
    ns = no // S
    assert no % S == 0

    nc = bacc.Bacc("TRN2", target_bir_lowering=False, debug=False,
                   num_devices=1)

    inp_t = nc.dram_tensor("inp_t", [K, nt], bf16, kind="ExternalInput")
    inp_own_t = nc.dram_tensor("inp_own_t", [K, no], bf16,
                               kind="ExternalInput")
    maskb = nc.dram_tensor("maskb", [nt, no], fp16, kind="ExternalInput")
    w_d = nc.dram_tensor("W", [K, F], f32, kind="ExternalInput")
    a_d = nc.dram_tensor("a", [2 * F, 1], f32, kind="ExternalInput")
    out_d = nc.dram_tensor("out", [no, F], f32, kind="ExternalOutput")

    # mask rows as [partition, chunk, i]: row (n*128 + p) -> [p, n]
    mask_r = maskb.ap().rearrange("(n p) i -> p n i", p=P)

    npair = ncj // 2
    n_act = int(round(act_frac * npair))
    n_gps = int(round(gps_frac * npair))

    def lrelu_engine(jp):
        return "act" if (jp * 797) % npair < n_act else "dve"

    def and_engine(jp):
        return "gps" if (jp * 509) % npair < n_gps else "dve"

    with tile.TileContext(nc) as tc, ExitStack() as ctx:
        consts = ctx.enter_context(tc.tile_pool(name="consts", bufs=1))
        scr_ps = ctx.enter_context(
            tc.tile_pool(name="scr_ps", bufs=1, space="PSUM"))
        p1b_ps = ctx.enter_context(
            tc.tile_pool(name="p1b_ps", bufs=2, space="PSUM"))

        # ---- phase 0: stationary [Wa2 | W] in bf16, Wa1 replicated ------
        w_sb = consts.tile([K, F], f32)
        nc.sync.dma_start(w_sb[:], w_d.ap())
        a_row = consts.tile([1, 2 * F], f32)
        nc.sync.dma_start(a_row[:], a_d.ap().rearrange("n o -> o n"))

        ones_sb = consts.tile([P, P], f32)
        nc.vector.memset(ones_sb[:], 1.0)
        a_rep = consts.tile([P, 2 * F], f32)
        a_rep_ps = scr_ps.tile([P, 2 * F], f32, tag="scr")
        nc.tensor.matmul(a_rep_ps[:], ones_sb[0:1, :], a_row[:],
                         start=True, stop=True)
        nc.vector.tensor_copy(a_rep[:], a_rep_ps[:])

        wa12_sb = consts.tile([K, 2], f32)
        wtmp = consts.tile([K, F], f32)
        nc.vector.tensor_tensor(wtmp[:], w_sb[:], a_rep[:, 0:F],
                                AluOpType.mult)
        nc.vector.tensor_reduce(wa12_sb[:, 0:1], wtmp[:],
                                mybir.AxisListType.X, AluOpType.add)
        nc.vector.tensor_tensor(wtmp[:], w_sb[:], a_rep[:, F:2 * F],
                                AluOpType.mult)
        nc.vector.tensor_reduce(wa12_sb[:, 1:2], wtmp[:],
                                mybir.AxisListType.X, AluOpType.add)

        stat_b = consts.tile([K, FE], bf16)        # [Wa2 | W] bf16
        nc.vector.tensor_copy(stat_b[:, 0:1], wa12_sb[:, 1:2])
        nc.vector.tensor_copy(stat_b[:, 1:FE], w_sb[:])
        wa1_rep = consts.tile([K, P], bf16)        # Wa1 replicated
        nc.vector.tensor_scalar(wa1_rep[:], ones_sb[:], wa12_sb[:, 0:1],
                                None, AluOpType.mult)

        # ---- phase 1a: wh1 replicated to all partitions (fp16) ----------
        io_b = consts.tile([K, no], bf16)
        nc.sync.dma_start(io_b[:], inp_own_t.ap())
        wh1_h = consts.tile([P, no], fp16)
        for s in range(ns):
            w1p = scr_ps.tile([P, S], f32, tag="scr")
            nc.tensor.matmul(w1p[:], wa1_rep[:], io_b[:, s * S:(s + 1) * S],
                             start=True, stop=True)
            nc.vector.tensor_copy(wh1_h[:, s * S:(s + 1) * S], w1p[:])

        # ---- phase 1b tables (filled per window below) ------------------
        h_ext = consts.tile([P, ncj, FE2], bf16)   # [wh2 | h | ones]
        wh2_sb = consts.tile([P, ncj], f32)        # Wh2 (f32, DVE scalar)
        wh2b_sb = consts.tile([P, ncj], f32)       # Wh2 * S (ACT bias)
        nc.vector.memset(h_ext[:, :, FE], 1.0)

        # ---- phase 2: main loop over j windows --------------------------
        pt_pool = ctx.enter_context(
            tc.tile_pool(name="pt_acc", bufs=1, space="PSUM"))
        pt_ps = pt_pool.tile([FE, no], f32)

        if ragged:
            sizes = [cpw] * ((ncj - 8) // cpw) + [4, 2, 2]
        else:
            sizes = [cpw] * (ncj // cpw)
        assert sum(sizes) == ncj and all(s % 2 == 0 for s in sizes)
        wins = []
        c0 = 0
        for sz in sizes:
            wins.append((c0, sz))
            c0 += sz
        nwin = len(wins)

        with (
            tc.tile_pool(name="aw", bufs=2) as aw_pool,
            tc.tile_pool(name="tin", bufs=2) as tin_pool,
            tc.tile_pool(name="lscr", bufs=4) as l_pool,
            tc.tile_pool(name="upool", bufs=3) as u_pool,
        ):
            aw_tiles = {}
            u_tiles = {}

            def issue_mask(wp):
                c0w, ncw = wins[wp]
                aw = aw_pool.tile([P, cpw, no], fp16, tag="aw",
                                  name=f"aw_{wp}")
                if wp <= 1:
                    # per-pair slices so the first ANDs start early
                    for jp in range(ncw // 2):
                        nc.gpsimd.dma_start(
                            aw[:, 2 * jp:2 * jp + 2, :],
                            mask_r[:, c0w + 2 * jp:c0w + 2 * jp + 2, :])
                else:
                    nc.gpsimd.dma_start(aw[:, 0:ncw, :],
                                        mask_r[:, c0w:c0w + ncw, :])
                aw_tiles[wp] = aw

            issue_mask(0)
            issue_mask(1)

            def prep_window(wp):
                c0w, ncw = wins[wp]
                if wp + 2 < nwin:
                    issue_mask(wp + 2)
                aw = aw_tiles.pop(wp)
                tp = tin_pool.tile([K, jw], bf16, tag="tin")
                nc.sync.dma_start(tp[:, 0:ncw * P],
                                  inp_t.ap()[:, c0w * P:(c0w + ncw) * P])
                # phase 1b: h/wh2 for these chunks (bf16 matmuls)
                for g0 in range(0, ncw, 4):
                    gn = min(4, ncw - g0)
                    hw_ps = p1b_ps.tile([P, 4, FE], f32, tag="p1b")
                    for q in range(gn):
                        nc.tensor.matmul(
                            hw_ps[:, q, :],
                            tp[:, (g0 + q) * P:(g0 + q + 1) * P],
                            stat_b[:], start=True, stop=True)
                    cs = slice(c0w + g0, c0w + g0 + gn)
                    nc.scalar.copy(h_ext[:, cs, 0:FE], hw_ps[:, 0:gn, :])
                    nc.vector.tensor_copy(wh2_sb[:, cs], hw_ps[:, 0:gn, 0])
                cs = slice(c0w, c0w + ncw)
                nc.vector.tensor_scalar(wh2b_sb[:, cs], wh2_sb[:, cs],
                                        KS, None, AluOpType.mult)
                # elementwise at PAIR (FD=3072) granularity: m3 = KS*lrelu(z)
                # (fp16, range +-3.5k), then ONE tt-add of the fp16
                # mask-and-bias tile awB {16248, 1100} with int16 output:
                # the f32 ALU computes m3 + awB and the int16 write IS the
                # Schraudolph exp bits, masked (masked bits < 4600 -> bf16
                # value < 2^-90).
                u_sb = u_pool.tile([P, cpw, no], bf16, tag="u",
                                   name=f"u_{wp}")
                u_bits = u_sb[:].bitcast(i16)
                for g0 in range(0, ncw, 4):
                    gn = min(4, ncw - g0)
                    m_t = l_pool.tile([P, 4, no], fp16, tag="lscr")
                    for jp2 in range(gn // 2):
                        jc0 = c0w + g0 + 2 * jp2
                        jp = jc0 // 2
                        if lrelu_engine(jp) == "act":
                            for q in range(2):
                                nc.scalar.activation(
                                    m_t[:, 2 * jp2 + q, :], wh1_h[:],
                                    AF.Prelu,
                                    bias=wh2b_sb[:, jc0 + q:jc0 + q + 1],
                                    scale=KS, alpha=ALPHA)
                        else:
                            z2 = l_pool.tile([P, 2, no], fp16, tag="lscr")
                            for q in range(2):
                                nc.vector.tensor_scalar(
                                    m_t[:, 2 * jp2 + q, :], wh1_h[:],
                                    wh2_sb[:, jc0 + q:jc0 + q + 1], KS,
                                    AluOpType.add, AluOpType.mult)
                                nc.vector.tensor_scalar(
                                    z2[:, q, :], wh1_h[:],
                                    wh2_sb[:, jc0 + q:jc0 + q + 1],
                                    ALPHA * KS,
                                    AluOpType.add, AluOpType.mult)
                            nc.vector.tensor_tensor(
                                m_t[:, 2 * jp2:2 * jp2 + 2, :],
                                m_t[:, 2 * jp2:2 * jp2 + 2, :], z2[:],
                                AluOpType.max)
                    nc.vector.tensor_tensor(
                        u_bits[:, g0:g0 + gn, :], m_t[:, 0:gn, :],
                        aw[:, g0:g0 + gn, :],
                        AluOpType.add)
                u_tiles[wp] = u_sb

            for wp in range(min(2, nwin)):
                prep_window(wp)
            next_prep = 2
            for w in range(nwin):
                c0w, ncw = wins[w]
                u_sb = u_tiles.pop(w)
                for q in range(ncw):
                    jc = c0w + q
                    for s in range(ns):
                        nc.tensor.matmul(
                            pt_ps[:, s * S:(s + 1) * S],
                            h_ext[:, jc, 1:FE2],
                            u_sb[:, q, s * S:(s + 1) * S],
                            start=(jc == 0),
                            stop=(jc == ncj - 1))
                if next_prep < nwin:
                    prep_window(next_prep)
                    next_prep += 1

        # ---- phase 3: out = elu(P[:, :64] / P[:, 64]), batched ----------
        from concourse.masks import make_identity
        with tc.tile_pool(name="fin_c", bufs=1) as fin_c, \
                tc.tile_pool(name="fin_ps", bufs=1, space="PSUM") as fin_ps:
            identity = fin_c.tile([P, P], f32)
            make_identity(nc, identity)
            pt_sb = fin_c.tile([FE, no], f32)
            nc.vector.tensor_copy(pt_sb[:], pt_ps[:])
            ptp = fin_ps.tile([P, nic, FE], f32)
            for ic in range(nic):
                nc.tensor.transpose(ptp[:, ic, :],
                                    pt_sb[:, ic * P:(ic + 1) * P],
                                    identity[0:FE, 0:FE])
            pts = fin_c.tile([P, nic, FE], f32)
            nc.scalar.copy(pts[:], ptp[:])
            rec = fin_c.tile([P, nic], f32)
            nc.vector.reciprocal(rec[:], pts[:, :, F])
            hp = fin_c.tile([P, nic, F], f32)
            for ic in range(nic):
                nc.vector.tensor_scalar(hp[:, ic, :], pts[:, ic, 0:F],
                                        rec[:, ic:ic + 1], None,
                                        AluOpType.mult)
            # elu(x) = max(x,0) + exp(min(x,0)) - 1
            mn = fin_c.tile([P, nic, F], f32)
            nc.vector.tensor_scalar(mn[:], hp[:], 0.0, None, AluOpType.min)
            nc.scalar.activation(mn[:], mn[:], AF.Exp)
            nc.vector.tensor_scalar(hp[:], hp[:], 0.0, None, AluOpType.max)
            ob = fin_c.tile([P, nic, F], f32)
            nc.vector.scalar_tensor_tensor(
                ob[:], mn[:], 1.0, hp[:],
                AluOpType.subtract, AluOpType.add)
            nc.gpsimd.dma_start(
                out_d.ap().rearrange("(ic p) f -> p ic f", p=P), ob[:])

    nc.compile()
    return nc


_CACHE = {}


def _get_program(nt, no, jw, **kw):
    key = (nt, no, jw, tuple(sorted(kw.items())))
    if key not in _CACHE:
        _CACHE[key] = build_program(nt, no, jw, **kw)
    return _CACHE[key]


def make_in_maps(input, adj, W, a):
    from concourse import mybir as _mb
    bf = _mb.dt.np(_mb.dt.bfloat16)
    input = np.ascontiguousarray(input, dtype=np.float32)
    adj = np.asarray(adj)
    W = np.ascontiguousarray(W, dtype=np.float32)
    a = np.ascontiguousarray(a, dtype=np.float32)
    nt = input.shape[0]
    no = nt // N_CORES
    inp_t = np.ascontiguousarray(input.T.astype(bf))
    # fp16 mask-and-bias: adj=1 -> BADD (bits offset), adj=0 -> BOFF
    mask_full = np.where(adj > 0, BADD, BOFF).astype(np.float16)
    in_maps = []
    for c in range(N_CORES):
        in_maps.append({
            "inp_t": inp_t,
            "inp_own_t": np.ascontiguousarray(inp_t[:, c * no:(c + 1) * no]),
            "maskb": np.ascontiguousarray(mask_full[c * no:(c + 1) * no].T),
            "W": W,
            "a": a,
        })
    return in_maps


def kernel(input, adj, W, a):
    from concourse.bass_utils import run_bass_kernel_spmd

    nt = input.shape[0]
    no = nt // N_CORES
    nc = _get_program(nt, no, JW)
    in_maps = make_in_maps(input, adj, W, a)
    res = run_bass_kernel_spmd(nc, in_maps, list(range(N_CORES)))
    return np.concatenate([r["out"] for r in res.results], axis=0)


# revision 30
# speedup vs baseline: 1.3551x; 1.3551x over previous
"""Attention graph convolution (GAT layer) on 8 TRN2 NeuronCores.

Reference computation (all fp32):
    h   = input @ W                      # (N, 64)
    e   = leakyrelu(h@a1 + (h@a2).T)     # (N, N)
    att = softmax(where(adj>0, e, -inf)) # row softmax
    out = elu(att @ h)                   # (N, 64)

Sharding: rows of e/att (= output rows) are split across 8 cores,
1536 rows each.  h (N x 64) is computed on every core (tiny).

Host-side prep: input transposed + cast bf16; adjacency row-block
transposed and expanded to an fp16 mask-and-bias tile
(adj=1 -> 16248.0, adj=0 -> 1100.0).  2-byte mask elements keep the
DVE combine op in a fast mode (1-byte operands drop DVE to 1x) and
plain (non-cast) DMAs run ~2x faster per byte than the cast DMAs the
int32 layout needed -- the mask stream is ~104us of DMA-engine time
vs ~198us for the baseline's int32 adjacency.

Per-core algorithm (core owns rows I, |I| = 1536; layout [j part, i free]):
  - m3 = KS*leakyrelu(Wh1[i] + Wh2[j]), KS = 2^7/ln2, computed per
    j-chunk either on ACT (one Prelu with scale=KS and per-partition
    bias KS*Wh2 -- 75% of chunk pairs) or on DVE (two 4x tensor_scalar
    + one tensor_tensor max).  m3 is fp16; |m3| < 3.6k so fp16 holds
    it to +-2 ulp.
  - exp+mask+bias in ONE DVE tensor_tensor per chunk pair:
    bits_i16 = m3 + awB, where awB is the DMA'd fp16 mask-and-bias
    tile.  The f32 ALU adds exactly and the int16-converting write
    yields Schraudolph exp bits: reinterpreted as bf16 they ARE
    exp(leakyrelu(z)) up to a +-4% sawtooth that cancels in the row
    softmax (normed output err ~4.5e-3, sim+HW validated).  Masked
    entries land at bits < 4600 -> bf16 value < 2^-90.  No exact exp
    anywhere in the main loop and no separate mask op.
  - all elementwise work happens in the window prep stage; the consume
    stage is a pure PE burst (24 bf16 matmuls accumulating [h|1].T @ U.T
    into f32 psum), which keeps the PE HAM clock-gate warm.
  - denominator via ones-column; out = elu(P[:, :64] / P[:, 64]).
"""

import numpy as np

N_TOTAL = 12288
K_IN = 128
F_OUT = 64
N_CORES = 8
ALPHA = 0.2
KS = 184.662545               # 2^7/ln2: bits-scale of lrelu(z)
BADD = 16248.0                # fp16-exact bits offset (127*2^7 - corr)
BOFF = 1100.0                 # masked-entry offset -> u < 2^-90
JW = 1024


def build_program(
    nt: int,          # total nodes
    no: int,          # nodes owned by this core
    jw: int,          # j window size
    act_frac: float = 0.75,   # fraction of j-chunk PAIRS with leakyrelu on ACT
    gps_frac: float = 0.0,    # gpsimd offload (Pool lacks the int16-out add)
    ragged: bool = True,
):
    from contextlib import ExitStack

    import concourse.bass as bass
    import concourse.mybir as mybir
    import concourse.tile as tile
    from concourse import bacc
    from concourse.alu_op_type import AluOpType

    f32 = mybir.dt.float32
    i16 = mybir.dt.int16
    bf16 = mybir.dt.bfloat16
    fp16 = mybir.dt.float16
    AF = mybir.ActivationFunctionType

    P = 128
    F = F_OUT
    FE = F + 1                    # [wh2 | h] phase-1b columns
    FE2 = FE + 1                  # h_ext columns: [wh2 | h | ones]
    K = K_IN
    assert nt % P == 0 and no % P == 0 and jw % P == 0 and nt % jw == 0
    ncj = nt // P                 # j chunks
    cpw = jw // P                 # j chunks per full window
    nic = no // P                 # i chunks (own rows)
    S = 512                       # psum bank split of the i dim
    ns = no // S
    assert no % S == 0

    nc = bacc.Bacc("TRN2", target_bir_lowering=False, debug=False,
                   num_devices=1)

    inp_t = nc.dram_tensor("inp_t", [K, nt], bf16, kind="ExternalInput")
    inp_own_t = nc.dram_tensor("inp_own_t", [K, no], bf16,
                               kind="ExternalInput")
    maskb = nc.dram_tensor("maskb", [nt, no], fp16, kind="ExternalInput")
    w_d = nc.dram_tensor("W", [K, F], f32, kind="ExternalInput")
    a_d = nc.dram_tensor("a", [2 * F, 1], f32, kind="ExternalInput")
    out_d = nc.dram_tensor("out", [no, F], f32, kind="ExternalOutput")

    # mask rows as [partition, chunk, i]: row (n*128 + p) -> [p, n]
    mask_r = maskb.ap().rearrange("(n p) i -> p n i", p=P)

    npair = ncj // 2
    n_act = int(round(act_frac * npair))
    n_gps = int(round(gps_frac * npair))

    def lrelu_engine(jp):
        return "act" if (jp * 797) % npair < n_act else "dve"

    def and_engine(jp):
        return "gps" if (jp * 509) % npair < n_gps else "dve"

    with tile.TileContext(nc) as tc, ExitStack() as ctx:
        consts = ctx.enter_context(tc.tile_pool(name="consts", bufs=1))
        scr_ps = ctx.enter_context(
            tc.tile_pool(name="scr_ps", bufs=1, space="PSUM"))
        p1b_ps = ctx.enter_context(
            tc.tile_pool(name="p1b_ps", bufs=2, space="PSUM"))

        # ---- phase 0: stationary [Wa2 | W] in bf16, Wa1 replicated ------
        w_sb = consts.tile([K, F], f32)
        nc.sync.dma_start(w_sb[:], w_d.ap())
        a_row = consts.tile([1, 2 * F], f32)
        nc.sync.dma_start(a_row[:], a_d.ap().rearrange("n o -> o n"))

        ones_sb = consts.tile([P, P], f32)
        nc.vector.memset(ones_sb[:], 1.0)
        a_rep = consts.tile([P, 2 * F], f32)
        a_rep_ps = scr_ps.tile([P, 2 * F], f32, tag="scr")
        nc.tensor.matmul(a_rep_ps[:], ones_sb[0:1, :], a_row[:],
                         start=True, stop=True)
        nc.vector.tensor_copy(a_rep[:], a_rep_ps[:])

        wa12_sb = consts.tile([K, 2], f32)
        wtmp = consts.tile([K, F], f32)
        nc.vector.tensor_tensor(wtmp[:], w_sb[:], a_rep[:, 0:F],
                                AluOpType.mult)
        nc.vector.tensor_reduce(wa12_sb[:, 0:1], wtmp[:],
                                mybir.AxisListType.X, AluOpType.add)
        nc.vector.tensor_tensor(wtmp[:], w_sb[:], a_rep[:, F:2 * F],
                                AluOpType.mult)
        nc.vector.tensor_reduce(wa12_sb[:, 1:2], wtmp[:],
                                mybir.AxisListType.X, AluOpType.add)

        stat_b = consts.tile([K, FE], bf16)        # [Wa2 | W] bf16
        nc.vector.tensor_copy(stat_b[:, 0:1], wa12_sb[:, 1:2])
        nc.vector.tensor_copy(stat_b[:, 1:FE], w_sb[:])
        wa1_rep = consts.tile([K, P], bf16)        # Wa1 replicated
        nc.vector.tensor_scalar(wa1_rep[:], ones_sb[:], wa12_sb[:, 0:1],
                                None, AluOpType.mult)

        # ---- phase 1a: wh1 replicated to all partitions (fp16) ----------
        io_b = consts.tile([K, no], bf16)
        nc.sync.dma_start(io_b[:], inp_own_t.ap())
        wh1_h = consts.tile([P, no], fp16)
        for s in range(ns):
            w1p = scr_ps.tile([P, S], f32, tag="scr")
            nc.tensor.matmul(w1p[:], wa1_rep[:], io_b[:, s * S:(s + 1) * S],
                             start=True, stop=True)
            nc.vector.tensor_copy(wh1_h[:, s * S:(s + 1) * S], w1p[:])

        # ---- phase 1b tables (filled per window below) ------------------
        h_ext = consts.tile([P, ncj, FE2], bf16)   # [wh2 | h | ones]
        wh2_sb = consts.tile([P, ncj], f32)        # Wh2 (f32, DVE scalar)
        wh2b_sb = consts.tile([P, ncj], f32)       # Wh2 * S (ACT bias)
        nc.vector.memset(h_ext[:, :, FE], 1.0)

        # ---- phase 2: main loop over j windows --------------------------
        pt_pool = ctx.enter_context(
            tc.tile_pool(name="pt_acc", bufs=1, space="PSUM"))
        pt_ps = pt_pool.tile([FE, no], f32)

        if ragged:
            sizes = [cpw] * ((ncj - 8) // cpw) + [4, 2, 2]
        else:
            sizes = [cpw] * (ncj // cpw)
        assert sum(sizes) == ncj and all(s % 2 == 0 for s in sizes)
        wins = []
        c0 = 0
        for sz in sizes:
            wins.append((c0, sz))
            c0 += sz
        nwin = len(wins)

        with (
            tc.tile_pool(name="aw", bufs=2) as aw_pool,
            tc.tile_pool(name="tin", bufs=2) as tin_pool,
            tc.tile_pool(name="lscr", bufs=4) as l_pool,
            tc.tile_pool(name="upool", bufs=3) as u_pool,
        ):
            aw_tiles = {}
            u_tiles = {}

            def issue_mask(wp):
                c0w, ncw = wins[wp]
                aw = aw_pool.tile([P, cpw, no], fp16, tag="aw",
                                  name=f"aw_{wp}")
                if wp <= 1:
                    # per-pair slices so the first ANDs start early
                    for jp in range(ncw // 2):
                        nc.gpsimd.dma_start(
                            aw[:, 2 * jp:2 * jp + 2, :],
                            mask_r[:, c0w + 2 * jp:c0w + 2 * jp + 2, :])
                else:
                    nc.gpsimd.dma_start(aw[:, 0:ncw, :],
                                        mask_r[:, c0w:c0w + ncw, :])
                aw_tiles[wp] = aw

            issue_mask(0)
            issue_mask(1)

            def prep_window(wp):
                c0w, ncw = wins[wp]
                if wp + 2 < nwin:
                    issue_mask(wp + 2)
                aw = aw_tiles.pop(wp)
                tp = tin_pool.tile([K, jw], bf16, tag="tin")
                nc.sync.dma_start(tp[:, 0:ncw * P],
                                  inp_t.ap()[:, c0w * P:(c0w + ncw) * P])
                # phase 1b: h/wh2 for these chunks (bf16 matmuls)
                for g0 in range(0, ncw, 4):
                    gn = min(4, ncw - g0)
                    hw_ps = p1b_ps.tile([P, 4, FE], f32, tag="p1b")
                    for q in range(gn):
                        nc.tensor.matmul(
                            hw_ps[:, q, :],
                            tp[:, (g0 + q) * P:(g0 + q + 1) * P],
                            stat_b[:], start=True, stop=True)
                    cs = slice(c0w + g0, c0w + g0 + gn)
                    nc.scalar.copy(h_ext[:, cs, 0:FE], hw_ps[:, 0:gn, :])
                    nc.vector.tensor_copy(wh2_sb[:, cs], hw_ps[:, 0:gn, 0])
                cs = slice(c0w, c0w + ncw)
                nc.vector.tensor_scalar(wh2b_sb[:, cs], wh2_sb[:, cs],
                                        KS, None, AluOpType.mult)
                # elementwise at PAIR (FD=3072) granularity: m3 = KS*lrelu(z)
                # (fp16, range +-3.5k), then ONE tt-add of the fp16
                # mask-and-bias tile awB {16248, 1100} with int16 output:
                # the f32 ALU computes m3 + awB and the int16 write IS the
                # Schraudolph exp bits, masked (masked bits < 4600 -> bf16
                # value < 2^-90).
                u_sb = u_pool.tile([P, cpw, no], bf16, tag="u",
                                   name=f"u_{wp}")
                u_bits = u_sb[:].bitcast(i16)
                for jp2 in range(ncw // 2):
                    jc0 = c0w + 2 * jp2
                    jp = jc0 // 2
                    m_t = l_pool.tile([P, 2, no], fp16, tag="lscr")
                    if lrelu_engine(jp) == "act":
                        for q in range(2):
                            nc.scalar.activation(
                                m_t[:, q, :], wh1_h[:], AF.Prelu,
                                bias=wh2b_sb[:, jc0 + q:jc0 + q + 1],
                                scale=KS, alpha=ALPHA)
                    else:
                        z2 = l_pool.tile([P, 2, no], fp16, tag="lscr")
                        for q in range(2):
                            nc.vector.tensor_scalar(
                                m_t[:, q, :], wh1_h[:],
                                wh2_sb[:, jc0 + q:jc0 + q + 1], KS,
                                AluOpType.add, AluOpType.mult)
                            nc.vector.tensor_scalar(
                                z2[:, q, :], wh1_h[:],
                                wh2_sb[:, jc0 + q:jc0 + q + 1], ALPHA * KS,
                                AluOpType.add, AluOpType.mult)
                        nc.vector.tensor_tensor(
                            m_t[:], m_t[:], z2[:], AluOpType.max)
                    eng = nc.gpsimd if and_engine(jp) == "gps" else nc.vector
                    eng.tensor_tensor(
                        u_bits[:, 2 * jp2:2 * jp2 + 2, :], m_t[:],
                        aw[:, 2 * jp2:2 * jp2 + 2, :],
                        AluOpType.add)
                u_tiles[wp] = u_sb

            for wp in range(min(2, nwin)):
                prep_window(wp)
            next_prep = 2
            for w in range(nwin):
                c0w, ncw = wins[w]
                u_sb = u_tiles.pop(w)
                for q in range(ncw):
                    jc = c0w + q
                    for s in range(ns):
                        nc.tensor.matmul(
                            pt_ps[:, s * S:(s + 1) * S],
                            h_ext[:, jc, 1:FE2],
                            u_sb[:, q, s * S:(s + 1) * S],
                            start=(jc == 0),
                            stop=(jc == ncj - 1))
                if next_prep < nwin:
                    prep_window(next_prep)
                    next_prep += 1

        # ---- phase 3: out = elu(P[:, :64] / P[:, 64]), batched ----------
        from concourse.masks import make_identity
        with tc.tile_pool(name="fin_c", bufs=1) as fin_c, \
                tc.tile_pool(name="fin_ps", bufs=1, space="PSUM") as fin_ps:
            identity = fin_c.tile([P, P], f32)
            make_identity(nc, identity)
            pt_sb = fin_c.tile([FE, no], f32)
            nc.vector.tensor_copy(pt_sb[:], pt_ps[:])
            ptp = fin_ps.tile([P, nic, FE], f32)
            for ic in range(nic):
                nc.tensor.transpose(ptp[:, ic, :],
                                    pt_sb[:, ic * P:(ic + 1) * P],
                                    identity[0:FE, 0:FE])
            pts = fin_c.tile([P, nic, FE], f32)
            nc.scalar.copy(pts[:], ptp[:])
            rec = fin_c.tile([P, nic], f32)
            nc.vector.reciprocal(rec[:], pts[:, :, F])
            hp = fin_c.tile([P, nic, F], f32)
            for ic in range(nic):
                nc.vector.tensor_scalar(hp[:, ic, :], pts[:, ic, 0:F],
                                        rec[:, ic:ic + 1], None,
                                        AluOpType.mult)
            # elu(x) = max(x,0) + exp(min(x,0)) - 1
            mn = fin_c.tile([P, nic, F], f32)
            nc.vector.tensor_scalar(mn[:], hp[:], 0.0, None, AluOpType.min)
            nc.scalar.activation(mn[:], mn[:], AF.Exp)
            nc.vector.tensor_scalar(hp[:], hp[:], 0.0, None, AluOpType.max)
            ob = fin_c.tile([P, nic, F], f32)
            nc.vector.scalar_tensor_tensor(
                ob[:], mn[:], 1.0, hp[:],
                AluOpType.subtract, AluOpType.add)
            nc.gpsimd.dma_start(
                out_d.ap().rearrange("(ic p) f -> p ic f", p=P), ob[:])

    nc.compile()
    return nc


_CACHE = {}


def _get_program(nt, no, jw, **kw):
    key = (nt, no, jw, tuple(sorted(kw.items())))
    if key not in _CACHE:
        _CACHE[key] = build_program(nt, no, jw, **kw)
    return _CACHE[key]


def make_in_maps(input, adj, W, a):
    from concourse import mybir as _mb
    bf = _mb.dt.np(_mb.dt.bfloat16)
    input = np.ascontiguousarray(input, dtype=np.float32)
    adj = np.asarray(adj)
    W = np.ascontiguousarray(W, dtype=np.float32)
    a = np.ascontiguousarray(a, dtype=np.float32)
    nt = input.shape[0]
    no = nt // N_CORES
    inp_t = np.ascontiguousarray(input.T.astype(bf))
    # fp16 mask-and-bias: adj=1 -> BADD (bits offset), adj=0 -> BOFF
    mask_full = np.where(adj > 0, BADD, BOFF).astype(np.float16)
    in_maps = []
    for c in range(N_CORES):
        in_maps.append({
            "inp_t": inp_t,
            "inp_own_t": np.ascontiguousarray(inp_t[:, c * no:(c + 1) * no]),
            "maskb": np.ascontiguousarray(mask_full[c * no:(c + 1) * no].T),
            "W": W,
            "a": a,
        })
    return in_maps


def kernel(input, adj, W, a):
    from concourse.bass_utils import run_bass_kernel_spmd

    nt = input.shape[0]
    no = nt // N_CORES
    nc = _get_program(nt, no, JW)
    in_maps = make_in_maps(input, adj, W, a)
    res = run_bass_kernel_spmd(nc, in_maps, list(range(N_CORES)))
    return np.concatenate([r["out"] for r in res.results], axis=0)
